# revision 20
# baseline (speedup 1.0000x reference)
"""Triangular matmul C = triu(triu(A) @ triu(B)) on 8 TRN2 NeuronCores.

N=4096 fp32, viewed as a 32x32 grid of 128x128 blocks; the MAC work is the
block-tetrahedron {I <= K <= J} (5984 blocks of 128^3).

Sharding is 2D over the output: column "phases" (512-wide J-groups) are
split into two classes CLS = {0,3,5,6} / {1,2,4,7} carrying exactly half
the MACs each; within a class, rows go to 4 cores per ROWS_TBL (sets found
by local search balancing per-core max(compute, DMA) — ~748 MAC-blocks and
~13 MB of HBM traffic per core).  Core c = (row set c%4, class c//4).

Numerics: operands are rounded to bf16 on the host and each block product
is a single bf16 matmul accumulating in fp32 PSUM (rel err ~2e-3 vs the
fp32 reference; the harness gate is 2e-2).  C is staged to fp16 in SBUF
and upcast on the host.

Schedule per core: phases BIG-FIRST (descending), rows descending within
a phase; row i's sweep is q = i..4p+3 ascending (start at q == i, stop at
q == 4p+3) into one PSUM bank, then a DVE fp32->fp16 eviction that
overlaps the following rows.  A and B are host-packed per core in EXACT
consumption order, so all loads are sequential-prefix chunks of two flat
SBUF tiles, issued just-in-time a few rows ahead (Tile attaches a
reader's dependency to every already-emitted writer of a tile, so
upfront issue would stall the first matmul on the last chunk).
Everything — loads AND C stores — runs on the single SP HWDGE ring in
program order: a second ring is starved by SDMA inter-queue arbitration,
and stores are emitted a couple of rows after their eviction copy so
they never stall the ring.  The first ~3 MB of loads plus a matmul
warmup burst (for the HAM clock gate) are emitted BEFORE the Switch
computed-goto dispatch — identical for every core — hiding the ~6 us
DMA arming latency and most of the dispatch under the NEFF's fixed
startup.

The kernel takes FULL (unsharded) inputs and returns the FULL output.
"""

import numpy as np

N = 4096
BLK = 128
NB = 32
PW = 512  # phase width in cols (4 blocks) = one fp32 PSUM bank
N_CORES = 8
MODE = "bf16x1-2d-v7"

CLS = [[0, 3, 5, 6], [1, 2, 4, 7]]
# Row sets per class (4 cores each), from the assignment optimizer.
# (Rows 28-31 in class 0 have no class-0 output and emit nothing there.)
ROWS_TBL = [
    [[3, 7, 9, 14, 17, 20, 22, 24], [0, 6, 8, 10, 25, 26, 28, 31],
     [4, 11, 12, 13, 15, 16, 18, 23], [1, 2, 5, 19, 21, 27, 29, 30]],
    [[0, 1, 12, 24, 25, 26, 29, 31], [2, 3, 8, 13, 21, 22, 27, 30],
     [5, 7, 11, 14, 15, 16, 18, 20], [4, 6, 9, 10, 17, 19, 23, 28]],
]

A_CHUNK = 36  # steady-state A chunk (slots, ~1.1 MB)
B_CHUNK = 4096  # steady-state B chunk (cols, ~1 MB)
# pre-Switch prefix boundaries (identical for every core).  Kept small:
# dependencies are whole-tile, so every matmul waits for ALL chunks
# emitted before it — the first real matmul waits on the whole preamble.
PRE_B = [1024, 2560]
PRE_A = [12]
N_WARM = 22  # dummy warmup matmuls (beat the HAM clock gate)


def _core_rs(c):
    return c % 4, c // 4


def _rows_of(c):
    r, s = _core_rs(c)
    return ROWS_TBL[s][r]


def _phases(c):
    """[(p, active_rows_desc)] in processing order: phases descending."""
    _, s = _core_rs(c)
    out = []
    for p in sorted(CLS[s], reverse=True):
        act = sorted((i for i in _rows_of(c) if i <= 4 * p + 3), reverse=True)
        if act:
            out.append((p, act))
    return out


def _strips_desc(p, m):
    """K-strips (q, col0, width_cols) of phase p, q descending to m."""
    out = []
    for q in range(4 * p + 3, m - 1, -1):
        c0 = max(4 * p, q) * BLK
        out.append((q, c0, (4 * p + 4) * BLK - c0))
    return out


def _b_layout(c):
    """bpack in global consumption order: phases big-first, strips q-desc.
    Returns ({(p, q): col offset}, total width)."""
    off, w = {}, 0
    for p, act in _phases(c):
        for q, _, wid in _strips_desc(p, act[-1]):
            off[(p, q)] = w
            w += wid
    return off, w


def _a_layout(c):
    """Packed-A slots in consumption order: rows descending (first phase
    order), K ascending within a row."""
    phs = _phases(c)
    kmax = 4 * phs[0][0] + 3
    rows = sorted(set(i for _, act in phs for i in act), reverse=True)
    slots = {}
    for i in rows:
        for q in range(i, kmax + 1):
            slots[(q, i)] = len(slots)
    return slots


def _c_layout(c):
    """Packed-C 512-col slots: {(p, i): slot}, contiguous per phase."""
    slots = {}
    for p, act in _phases(c):
        for i in act:
            slots[(p, i)] = len(slots)
    return slots


NA_MAX = max(len(_a_layout(c)) for c in range(N_CORES))
WB_MAX = max(_b_layout(c)[1] for c in range(N_CORES))
NC_MAX = max(len(_c_layout(c)) for c in range(N_CORES))


def _emit_preamble(nc, tc, pools, dram_io):
    """Identical for all cores, before the Switch: PE warmup + the first
    load chunks, hiding DMA arming latency under the NEFF startup."""
    import concourse.mybir as mybir

    f32 = mybir.dt.float32
    bf16 = mybir.dt.bfloat16
    apool, bpool, cpool, psum_pool = pools
    apack, bpack = dram_io["apack"], dram_io["bpack"]

    a_t = apool.tile([BLK, NA_MAX, BLK], bf16, name="a", tag="a")
    b_t = bpool.tile([BLK, WB_MAX], bf16, name="b", tag="b")
    warm = bpool.tile([BLK, PW], bf16, name="warm", tag="warm")
    nc.gpsimd.memset(warm[:], 0.0)
    wps = psum_pool.tile([BLK, PW], f32, name="warmps", tag="ps7")
    for i in range(N_WARM):
        nc.tensor.matmul(
            wps[:], warm[:, :BLK], warm[:], start=(i == 0), stop=(i == N_WARM - 1)
        )
    lo_b = 0
    lo_a = 0
    for k in range(max(len(PRE_B), len(PRE_A))):
        if k < len(PRE_B):
            nc.sync.dma_start(b_t[:, lo_b : PRE_B[k]], bpack[:, lo_b : PRE_B[k]])
            lo_b = PRE_B[k]
        if k < len(PRE_A):
            nc.sync.dma_start(
                a_t[:, lo_a : PRE_A[k], :], apack[:, lo_a : PRE_A[k], :]
            )
            lo_a = PRE_A[k]
    return a_t, b_t


def _emit_core(nc, tc, pools, dram_io, core, a_t, b_t):
    import concourse.mybir as mybir

    f32 = mybir.dt.float32
    fp16 = mybir.dt.float16
    apool, bpool, cpool, psum_pool = pools
    apack, bpack, cpack = dram_io["apack"], dram_io["bpack"], dram_io["cpack"]
    aslot = _a_layout(core)
    cslot = _c_layout(core)
    boff, wb = _b_layout(core)
    phs = _phases(core)
    na = len(aslot)

    # sequential chunk issue beyond the preamble prefix
    a_done = PRE_A[-1]
    b_done = PRE_B[-1]

    def need(b_hi, a_hi):
        nonlocal a_done, b_done
        while b_done < min(b_hi + 1, wb):
            step = min(B_CHUNK, wb - b_done)
            nc.sync.dma_start(
                b_t[:, b_done : b_done + step], bpack[:, b_done : b_done + step]
            )
            b_done += step
        while a_done < min(a_hi + 1, na):
            step = min(A_CHUNK, na - a_done)
            nc.sync.dma_start(
                a_t[:, a_done : a_done + step, :],
                apack[:, a_done : a_done + step, :],
            )
            a_done += step

    seq = [(pi, p, act, i) for pi, (p, act) in enumerate(phs) for i in act]

    # compute: phases big-first, rows descending, per-row eviction.  Each
    # row's chunks are emitted at its own top (no lookahead): deps are
    # whole-tile, so any chunk emitted earlier than needed would stall the
    # current matmuls; the SP sequencer itself provides the run-ahead.
    # Stores stay OFF the load ring (a waiting store stalls it): cst
    # phase batches go out on SWDGE, the tiny final phase per-row on the
    # otherwise-idle ACT ring.
    bank = 0
    cst = {}
    for j, (pi, p, act, i) in enumerate(seq):
        b_hi = max(boff[(p, q)] + wid - 1 for q, _, wid in _strips_desc(p, i))
        a_hi = max(aslot[(q, i)] for q, _, _ in _strips_desc(p, i))
        need(b_hi, a_hi)
        last_strip = 4 * p + 3
        last_phase = pi == len(phs) - 1
        if not last_phase and p not in cst:
            cst[p] = cpool.tile(
                [BLK, len(act) * PW], fp16, name=f"cst_{p}", tag=f"cst{pi}"
            )
        pst = psum_pool.tile([BLK, PW], f32, name=f"ps_{p}_{i}", tag=f"ps{bank % 8}")
        bank += 1
        for q, c0, wid in reversed(_strips_desc(p, i)):
            rel = c0 - 4 * p * BLK
            nc.tensor.matmul(
                pst[:, rel : rel + wid],
                a_t[:, aslot[(q, i)], :],
                b_t[:, boff[(p, q)] : boff[(p, q)] + wid],
                start=(q == i),
                stop=(q == last_strip),
            )
        mr = max(0, i - 4 * p) * BLK
        if last_phase:
            ji = act.index(i)
            ct = cpool.tile([BLK, PW], fp16, name=f"ct_{i}", tag=f"ct{ji % 4}")
            nc.vector.tensor_copy(ct[:, mr:PW], pst[:, mr:PW])
            nc.scalar.dma_start(
                cpack[:, cslot[(p, i)] * PW + mr : (cslot[(p, i)] + 1) * PW],
                ct[:, mr:PW],
            )
        else:
            s0 = cslot[(p, act[0])]
            jrow = cslot[(p, i)] - s0
            nc.vector.tensor_copy(
                cst[p][:, jrow * PW + mr : (jrow + 1) * PW], pst[:, mr:PW]
            )
            if i == act[-1]:
                nc.gpsimd.dma_start(
                    cpack[:, s0 * PW : (s0 + len(act)) * PW], cst[p][:]
                )


def _build():
    import concourse.mybir as mybir
    import concourse.tile as tile
    from concourse import bacc

    nc = bacc.Bacc(None, target_bir_lowering=False, debug=False)
    bf16 = mybir.dt.bfloat16
    fp16 = mybir.dt.float16
    with tile.TileContext(nc) as tc:
        with (
            tc.tile_pool(name="dram", bufs=1, space="DRAM") as dram,
            tc.tile_pool(name="apool", bufs=1) as apool,
            tc.tile_pool(name="bpool", bufs=1) as bpool,
            tc.tile_pool(name="cpool", bufs=1) as cpool,
            tc.tile_pool(name="psum", bufs=1, space="PSUM") as psum_pool,
        ):
            dram_io = {
                "apack": dram.tile(
                    [BLK, NA_MAX, BLK], bf16, kind="ExternalInput",
                    name="apack", uniquify=False,
                ),
                "bpack": dram.tile(
                    [BLK, WB_MAX], bf16, kind="ExternalInput",
                    name="bpack", uniquify=False,
                ),
                "cpack": dram.tile(
                    [BLK, NC_MAX * PW], fp16, kind="ExternalOutput",
                    name="cpack", uniquify=False,
                ),
            }
            pid = nc.partition_id()
            pools = (apool, bpool, cpool, psum_pool)
            arm_engines = [
                e for e in mybir.ALL_ENGINES if e.name in ("SP", "PE", "DVE")
            ]
            tc.switch_hint({e: pid for e in arm_engines}, N_CORES, label="core")
            a_t, b_t = _emit_preamble(nc, tc, pools, dram_io)
            for c in tc.Switch(pid, N_CORES, hint="core"):
                _emit_core(nc, tc, pools, dram_io, c, a_t, b_t)
    nc.compile()
    return nc


_cached_nc = None

# Optional profiling knobs (used by test.py; harness leaves them off).
TRACE = False
TRACE_KW = {}
LAST_RESULTS = None


def _get_nc():
    global _cached_nc
    if _cached_nc is None:
        _cached_nc = _build()
    return _cached_nc


def _host_pack(A, B):
    import ml_dtypes

    bf16 = ml_dtypes.bfloat16
    AT = np.ascontiguousarray(A.T).astype(bf16)
    Bb = B.astype(bf16)
    apacks, bpacks = [], []
    for c in range(N_CORES):
        ap = np.zeros((BLK, NA_MAX, BLK), dtype=bf16)
        for (q, i), idx in _a_layout(c).items():
            ap[:, idx, :] = AT[q * BLK : (q + 1) * BLK, i * BLK : (i + 1) * BLK]
        bp = np.zeros((BLK, WB_MAX), dtype=bf16)
        boff, _ = _b_layout(c)
        for p, act in _phases(c):
            for q, c0, wid in _strips_desc(p, act[-1]):
                w0 = boff[(p, q)]
                bp[:, w0 : w0 + wid] = Bb[q * BLK : (q + 1) * BLK, c0 : c0 + wid]
        apacks.append(ap)
        bpacks.append(bp)
    return apacks, bpacks


def kernel(A, B):
    from concourse.bass_utils import run_bass_kernel_spmd

    A = np.asarray(A, dtype=np.float32)
    B = np.asarray(B, dtype=np.float32)
    nc = _get_nc()
    apacks, bpacks = _host_pack(A, B)
    in_maps = [{"apack": apacks[c], "bpack": bpacks[c]} for c in range(N_CORES)]
    res = run_bass_kernel_spmd(
        nc, in_maps, core_ids=list(range(N_CORES)), trace=TRACE, **TRACE_KW
    )
    global LAST_RESULTS
    LAST_RESULTS = res

    C = np.zeros((N, N), dtype=np.float32)
    for c in range(N_CORES):
        cp = res.results[c]["cpack"]
        for (p, i), j in _c_layout(c).items():
            mr = max(0, i - 4 * p) * BLK
            C[i * BLK : (i + 1) * BLK, p * PW + mr : (p + 1) * PW] = cp[
                :, j * PW + mr : (j + 1) * PW
            ].astype(np.float32)
    return np.triu(C)
